# revision 9
# baseline (speedup 1.0000x reference)
"""PointNet-style kernel for Trainium2, sharded across 8 NeuronCores.

Math note: reference computes score = [feat | hidden | seg_tiled] @ cW + cb,
then per-bbox rescales (s - min) / (max - min) * 2 - 1.  The seg-MLP
contribution and cb are constant within a bbox, and the rescale is invariant
to per-bbox additive constants, so they cancel exactly.  Only the embed MLP
and the per-point part of the classifier affect the output:
    s_pt[m, n] = feat[m,n,:] @ cW[0:6] + h3[m,n,:] @ cW[6:262]

Sharding: pure data-parallel over the M (bbox) axis - 8 bboxes per core.

Device layout per core: activations are kept channels-on-partitions,
points-streaming ([C, Npts] tiles of 512 points).  Matmuls run in float32r
(fp32 storage, reduced-precision multiply, 4x faster than fp32 on the PE,
~1.5e-4 rel err per matmul as measured on hardware).
"""

import numpy as np

import concourse.bass as bass
import concourse.mybir as mybir
import concourse.tile as tile
from concourse import bacc
from concourse.bass_utils import run_bass_kernel_spmd

N_CORES = 8
B = 8            # bboxes per core
NPTS = 8192      # points per bbox
IN_DIM = 6
HID = 256
TN = 512         # points per tile
NT = NPTS // TN  # 16 tiles per bbox

f32 = mybir.dt.float32
f32r = mybir.dt.float32r
RELU = mybir.ActivationFunctionType.Relu
ADD = mybir.AluOpType.add
MAX = mybir.AluOpType.max
MIN = mybir.AluOpType.min
MULT = mybir.AluOpType.mult
SUB = mybir.AluOpType.subtract
AX = mybir.AxisListType.X

_CACHE = {}


def _build_program():
    nc = bacc.Bacc("TRN2", target_bir_lowering=False, debug=False)

    xt = nc.dram_tensor("xt", [B, IN_DIM, NPTS], f32r, kind="ExternalInput")
    w1 = nc.dram_tensor("w1", [IN_DIM, HID], f32r, kind="ExternalInput")
    w2p = nc.dram_tensor("w2p", [128, 512], f32r, kind="ExternalInput")
    w3p = nc.dram_tensor("w3p", [128, 512], f32r, kind="ExternalInput")
    # classifier weights with masked columns: for tile t, column t (of 16) is
    # live and the rest are zero, so all 16 tiles of a bbox accumulate into
    # one [16, 512] PSUM bank (row t = tile t's scores)
    cw6m = nc.dram_tensor("cw6m", [IN_DIM, NT * NT], f32r, kind="ExternalInput")
    cwhm = nc.dram_tensor("cwhm", [128, NT * 2 * NT], f32r, kind="ExternalInput")
    b1d = nc.dram_tensor("b1", [128, 2], f32, kind="ExternalInput")
    b2d = nc.dram_tensor("b2", [128, 2], f32, kind="ExternalInput")
    b3d = nc.dram_tensor("b3", [128, 2], f32, kind="ExternalInput")
    score_d = nc.dram_tensor("score", [B, NT, TN], f32, kind="ExternalOutput")

    with (
        tile.TileContext(nc) as tc,
        tc.tile_pool(name="wp", bufs=1) as wp,
        tc.tile_pool(name="xp", bufs=4) as xp,
        tc.tile_pool(name="hp", bufs=3) as hp,
        tc.tile_pool(name="sp", bufs=1) as sp,
        tc.tile_pool(name="st", bufs=1) as st,
        tc.tile_pool(name="dsc", bufs=1, space="DRAM") as dsc,
        tc.tile_pool(name="pp", bufs=1, space="PSUM") as pp,
    ):
        w1_t = wp.tile([IN_DIM, HID], f32r, tag="w1")
        nc.sync.dma_start(out=w1_t[:], in_=w1[:])
        w2_t = wp.tile([128, 512], f32r, tag="w2")
        nc.sync.dma_start(out=w2_t[:], in_=w2p[:])
        w3_t = wp.tile([128, 512], f32r, tag="w3")
        nc.sync.dma_start(out=w3_t[:], in_=w3p[:])
        cw6_t = wp.tile([IN_DIM, NT * NT], f32r, tag="cw6")
        nc.sync.dma_start(out=cw6_t[:], in_=cw6m[:])
        cwh_t = wp.tile([128, NT * 2 * NT], f32r, tag="cwh")
        nc.sync.dma_start(out=cwh_t[:], in_=cwhm[:])
        b1_t = wp.tile([128, 2], f32, tag="b1")
        nc.sync.dma_start(out=b1_t[:], in_=b1d[:])
        b2_t = wp.tile([128, 2], f32, tag="b2")
        nc.sync.dma_start(out=b2_t[:], in_=b2d[:])
        b3_t = wp.tile([128, 2], f32, tag="b3")
        nc.sync.dma_start(out=b3_t[:], in_=b3d[:])

        # raw per-point scores: partition p = b*NT + t holds points
        # [t*TN, (t+1)*TN) of bbox b
        score_sb = sp.tile([128, TN], f32, tag="score")

        for b in range(B):
            psc = pp.tile([NT, TN], f32, tag="psc", bufs=2)
            for t in range(NT):
                xt_t = xp.tile([IN_DIM, TN], f32r, tag="xt")
                nc.sync.dma_start(out=xt_t[:], in_=xt[b, :, t * TN:(t + 1) * TN])

                # layer 1: [6 -> 256]
                pa0 = pp.tile([128, TN], f32, tag="lay", bufs=6)
                pa1 = pp.tile([128, TN], f32, tag="lay", bufs=6)
                nc.tensor.matmul(pa0[:], w1_t[:, 0:128], xt_t[:], start=True, stop=True)
                nc.tensor.matmul(pa1[:], w1_t[:, 128:256], xt_t[:], start=True, stop=True)
                h10 = hp.tile([128, TN], f32r, tag="h10")
                h11 = hp.tile([128, TN], f32r, tag="h11")
                nc.scalar.activation(out=h10[:], in_=pa0[:], func=RELU, bias=b1_t[:, 0:1], scale=1.0)
                nc.vector.tensor_scalar(
                    out=h11[:], in0=pa1[:], scalar1=b1_t[:, 1:2], scalar2=0.0, op0=ADD, op1=MAX
                )

                # layer 2: [256 -> 256]
                pb0 = pp.tile([128, TN], f32, tag="lay", bufs=6)
                pb1 = pp.tile([128, TN], f32, tag="lay", bufs=6)
                nc.tensor.matmul(pb0[:], w2_t[:, 0:128], h10[:], start=True, stop=False)
                nc.tensor.matmul(pb0[:], w2_t[:, 256:384], h11[:], start=False, stop=True)
                nc.tensor.matmul(pb1[:], w2_t[:, 128:256], h10[:], start=True, stop=False)
                nc.tensor.matmul(pb1[:], w2_t[:, 384:512], h11[:], start=False, stop=True)
                h20 = hp.tile([128, TN], f32r, tag="h20")
                h21 = hp.tile([128, TN], f32r, tag="h21")
                nc.scalar.activation(out=h20[:], in_=pb0[:], func=RELU, bias=b2_t[:, 0:1], scale=1.0)
                nc.vector.tensor_scalar(
                    out=h21[:], in0=pb1[:], scalar1=b2_t[:, 1:2], scalar2=0.0, op0=ADD, op1=MAX
                )

                # layer 3: [256 -> 256]
                pc0 = pp.tile([128, TN], f32, tag="lay", bufs=6)
                pc1 = pp.tile([128, TN], f32, tag="lay", bufs=6)
                nc.tensor.matmul(pc0[:], w3_t[:, 0:128], h20[:], start=True, stop=False)
                nc.tensor.matmul(pc0[:], w3_t[:, 256:384], h21[:], start=False, stop=True)
                nc.tensor.matmul(pc1[:], w3_t[:, 128:256], h20[:], start=True, stop=False)
                nc.tensor.matmul(pc1[:], w3_t[:, 384:512], h21[:], start=False, stop=True)
                h30 = hp.tile([128, TN], f32r, tag="h30")
                h31 = hp.tile([128, TN], f32r, tag="h31")
                nc.scalar.activation(out=h30[:], in_=pc0[:], func=RELU, bias=b3_t[:, 0:1], scale=1.0)
                nc.vector.tensor_scalar(
                    out=h31[:], in0=pc1[:], scalar1=b3_t[:, 1:2], scalar2=0.0, op0=ADD, op1=MAX
                )

                # classifier per-point part: feat @ cW[0:6] + h3 @ cW[6:262]
                # masked-column weights put tile t's scores in psum row t
                nc.tensor.matmul(
                    psc[:], cw6_t[:, t * NT:(t + 1) * NT], xt_t[:],
                    start=(t == 0), stop=False)
                nc.tensor.matmul(
                    psc[:], cwh_t[:, t * 32:t * 32 + 16], h30[:],
                    start=False, stop=False)
                nc.tensor.matmul(
                    psc[:], cwh_t[:, t * 32 + 16:t * 32 + 32], h31[:],
                    start=False, stop=(t == NT - 1))

            # drain the bbox's [16, 512] scores: PSUM -> SBUF staging -> DMA
            stg = hp.tile([NT, TN], f32, tag="stg", bufs=2)
            if b % 2 == 0:
                nc.scalar.copy(out=stg[:], in_=psc[:])
            else:
                nc.vector.tensor_copy(out=stg[:], in_=psc[:])
            nc.sync.dma_start(out=score_sb[b * NT:(b + 1) * NT, :], in_=stg[:])

        # per-bbox min/max rescale tail
        mn = st.tile([128, 1], f32, tag="mn")
        mx = st.tile([128, 1], f32, tag="mx")
        nc.vector.tensor_reduce(out=mn[:], in_=score_sb[:], axis=AX, op=MIN)
        nc.vector.tensor_reduce(out=mx[:], in_=score_sb[:], axis=AX, op=MAX)

        # regroup [128,1] -> [8,16] via DRAM so each bbox's partials share a row
        mnd = dsc.tile([128], f32, tag="mnd")
        mxd = dsc.tile([128], f32, tag="mxd")
        nc.sync.dma_start(out=mnd[:], in_=mn[:])
        nc.sync.dma_start(out=mxd[:], in_=mx[:])
        mn8 = st.tile([B, NT], f32, tag="mn8")
        mx8 = st.tile([B, NT], f32, tag="mx8")
        nc.sync.dma_start(out=mn8[:], in_=mnd[:].rearrange("(a c) -> a c", a=B))
        nc.sync.dma_start(out=mx8[:], in_=mxd[:].rearrange("(a c) -> a c", a=B))
        mnb = st.tile([B, 1], f32, tag="mnb")
        mxb = st.tile([B, 1], f32, tag="mxb")
        nc.vector.tensor_reduce(out=mnb[:], in_=mn8[:], axis=AX, op=MIN)
        nc.vector.tensor_reduce(out=mxb[:], in_=mx8[:], axis=AX, op=MAX)

        # k2 = 2/(max-min); off = -min*k2 - 1
        rngb = st.tile([B, 1], f32, tag="rngb")
        nc.vector.tensor_tensor(out=rngb[:], in0=mxb[:], in1=mnb[:], op=SUB)
        invb = st.tile([B, 1], f32, tag="invb")
        nc.vector.reciprocal(out=invb[:], in_=rngb[:])
        k2b = st.tile([B, 1], f32, tag="k2b")
        nc.vector.tensor_scalar(
            out=k2b[:], in0=invb[:], scalar1=2.0, scalar2=None, op0=MULT
        )
        tmpb = st.tile([B, 1], f32, tag="tmpb")
        nc.vector.tensor_tensor(out=tmpb[:], in0=mnb[:], in1=k2b[:], op=MULT)
        offb = st.tile([B, 1], f32, tag="offb")
        nc.vector.tensor_scalar(
            out=offb[:], in0=tmpb[:], scalar1=-1.0, scalar2=-1.0, op0=MULT, op1=ADD
        )

        # broadcast [8,1] -> [128,1] (partition p gets value for bbox p//NT)
        k2d = dsc.tile([B], f32, tag="k2d")
        offd = dsc.tile([B], f32, tag="offd")
        nc.sync.dma_start(out=k2d[:], in_=k2b[:])
        nc.sync.dma_start(out=offd[:], in_=offb[:])
        k2f = st.tile([128, 1], f32, tag="k2f")
        offf = st.tile([128, 1], f32, tag="offf")
        k2d_ap = k2d[:]
        offd_ap = offd[:]
        nc.sync.dma_start(
            out=k2f[:],
            in_=bass.AP(tensor=k2d_ap.tensor, offset=k2d_ap.offset, ap=[[1, B], [0, NT]]),
        )
        nc.sync.dma_start(
            out=offf[:],
            in_=bass.AP(tensor=offd_ap.tensor, offset=offd_ap.offset, ap=[[1, B], [0, NT]]),
        )

        final = sp.tile([128, TN], f32, tag="final")
        nc.vector.tensor_scalar(
            out=final[:], in0=score_sb[:], scalar1=k2f[:], scalar2=offf[:], op0=MULT, op1=ADD
        )
        nc.sync.dma_start(out=score_d[:], in_=final[:])

    nc.finalize()
    return nc


def _get_program():
    if "nc" not in _CACHE:
        _CACHE["nc"] = _build_program()
    return _CACHE["nc"]


def _make_in_maps(np_inputs):
    feat = np.asarray(np_inputs["feat"], dtype=np.float32)
    eW1 = np.asarray(np_inputs["eW1"], dtype=np.float32)
    eb1 = np.asarray(np_inputs["eb1"], dtype=np.float32)
    eW2 = np.asarray(np_inputs["eW2"], dtype=np.float32)
    eb2 = np.asarray(np_inputs["eb2"], dtype=np.float32)
    eW3 = np.asarray(np_inputs["eW3"], dtype=np.float32)
    eb3 = np.asarray(np_inputs["eb3"], dtype=np.float32)
    cW = np.asarray(np_inputs["cW"], dtype=np.float32)

    # [64, 8192, 6] -> [64, 6, 8192], channel-major per bbox
    xt_all = np.ascontiguousarray(np.transpose(feat, (0, 2, 1)))

    def pack_w(w):  # [256,256] -> [128, 512] blocks [k0m0 | k0m1 | k1m0 | k1m1]
        return np.ascontiguousarray(np.concatenate(
            [w[0:128, 0:128], w[0:128, 128:256], w[128:256, 0:128], w[128:256, 128:256]],
            axis=1))

    # masked classifier weights: tile t's live column is t
    cw6m = np.zeros((IN_DIM, NT, NT), dtype=np.float32)
    cwhm = np.zeros((128, NT, 2 * NT), dtype=np.float32)
    for t in range(NT):
        cw6m[:, t, t] = cW[0:IN_DIM, 0]
        cwhm[:, t, t] = cW[6:134, 0]
        cwhm[:, t, NT + t] = cW[134:262, 0]

    common = {
        "w1": np.ascontiguousarray(eW1),
        "w2p": pack_w(eW2),
        "w3p": pack_w(eW3),
        "cw6m": np.ascontiguousarray(cw6m.reshape(IN_DIM, NT * NT)),
        "cwhm": np.ascontiguousarray(cwhm.reshape(128, NT * 2 * NT)),
        "b1": np.ascontiguousarray(np.stack([eb1[0:128], eb1[128:256]], axis=1)),
        "b2": np.ascontiguousarray(np.stack([eb2[0:128], eb2[128:256]], axis=1)),
        "b3": np.ascontiguousarray(np.stack([eb3[0:128], eb3[128:256]], axis=1)),
    }
    return [
        {"xt": np.ascontiguousarray(xt_all[c * B:(c + 1) * B]), **common}
        for c in range(N_CORES)
    ]


def kernel(feat, eW1, eb1, eW2, eb2, eW3, eb3,
           sW1, sb1, sW2, sb2, sW3, sb3, cW, cb):
    nc = _get_program()
    in_maps = _make_in_maps({
        "feat": feat, "eW1": eW1, "eb1": eb1, "eW2": eW2, "eb2": eb2,
        "eW3": eW3, "eb3": eb3, "cW": cW,
    })
    res = run_bass_kernel_spmd(nc, in_maps, list(range(N_CORES))).results
    out = np.concatenate(
        [np.asarray(res[c]["score"]).reshape(B, NPTS) for c in range(N_CORES)], axis=0
    )
    return np.ascontiguousarray(out.astype(np.float32))


# revision 11
# speedup vs baseline: 1.2245x; 1.2245x over previous
"""PointNet-style kernel for Trainium2, sharded across 8 NeuronCores.

Math note: reference computes score = [feat | hidden | seg_tiled] @ cW + cb,
then per-bbox rescales (s - min) / (max - min) * 2 - 1.  The seg-MLP
contribution and cb are constant within a bbox, and the rescale is invariant
to per-bbox additive constants, so they cancel exactly.  Only the embed MLP
and the per-point part of the classifier affect the output:
    s_pt[m, n] = feat[m,n,:] @ cW[0:6] + h3[m,n,:] @ cW[6:262]

Sharding: pure data-parallel over the M (bbox) axis - 8 bboxes per core.

Device layout per core: activations are kept channels-on-partitions,
points-streaming ([C, Npts] tiles of 512 points).  Matmuls run in float32r
(fp32 storage, reduced-precision multiply, 4x faster than fp32 on the PE,
~1.5e-4 rel err per matmul as measured on hardware).
"""

import numpy as np

import concourse.bass as bass
import concourse.mybir as mybir
import concourse.tile as tile
from concourse import bacc
from concourse.bass_utils import run_bass_kernel_spmd

N_CORES = 8
B = 8            # bboxes per core
NPTS = 8192      # points per bbox
IN_DIM = 6
HID = 256
TN = 512         # points per tile
NT = NPTS // TN  # 16 tiles per bbox

f32 = mybir.dt.float32
f32r = mybir.dt.float32r
RELU = mybir.ActivationFunctionType.Relu
ADD = mybir.AluOpType.add
MAX = mybir.AluOpType.max
MIN = mybir.AluOpType.min
MULT = mybir.AluOpType.mult
SUB = mybir.AluOpType.subtract
AX = mybir.AxisListType.X

_CACHE = {}


def _build_program():
    nc = bacc.Bacc("TRN2", target_bir_lowering=False, debug=False)

    xt = nc.dram_tensor("xt", [B, IN_DIM, NPTS], f32r, kind="ExternalInput")
    w1 = nc.dram_tensor("w1", [IN_DIM, HID], f32r, kind="ExternalInput")
    w2p = nc.dram_tensor("w2p", [128, 512], f32r, kind="ExternalInput")
    w3p = nc.dram_tensor("w3p", [128, 512], f32r, kind="ExternalInput")
    # classifier weights with masked columns: for tile t, column t (of 16) is
    # live and the rest are zero, so all 16 tiles of a bbox accumulate into
    # one [16, 512] PSUM bank (row t = tile t's scores)
    cw6m = nc.dram_tensor("cw6m", [IN_DIM, NT * NT], f32r, kind="ExternalInput")
    cwhm = nc.dram_tensor("cwhm", [128, NT * 2 * NT], f32r, kind="ExternalInput")
    b1d = nc.dram_tensor("b1", [128, 2], f32, kind="ExternalInput")
    b2d = nc.dram_tensor("b2", [128, 2], f32, kind="ExternalInput")
    b3d = nc.dram_tensor("b3", [128, 2], f32, kind="ExternalInput")
    score_d = nc.dram_tensor("score", [B, NT, TN], f32, kind="ExternalOutput")

    with (
        tile.TileContext(nc) as tc,
        tc.tile_pool(name="wp", bufs=1) as wp,
        tc.tile_pool(name="xp", bufs=4) as xp,
        tc.tile_pool(name="hp", bufs=3) as hp,
        tc.tile_pool(name="sp", bufs=1) as sp,
        tc.tile_pool(name="st", bufs=1) as st,
        tc.tile_pool(name="dsc", bufs=1, space="DRAM") as dsc,
        tc.tile_pool(name="pp", bufs=1, space="PSUM") as pp,
    ):
        w1_t = wp.tile([IN_DIM, HID], f32r, tag="w1")
        nc.sync.dma_start(out=w1_t[:], in_=w1[:])
        w2_t = wp.tile([128, 512], f32r, tag="w2")
        nc.sync.dma_start(out=w2_t[:], in_=w2p[:])
        w3_t = wp.tile([128, 512], f32r, tag="w3")
        nc.sync.dma_start(out=w3_t[:], in_=w3p[:])
        cw6_t = wp.tile([IN_DIM, NT * NT], f32r, tag="cw6")
        nc.sync.dma_start(out=cw6_t[:], in_=cw6m[:])
        cwh_t = wp.tile([128, NT * 2 * NT], f32r, tag="cwh")
        nc.sync.dma_start(out=cwh_t[:], in_=cwhm[:])
        b1_t = wp.tile([128, 2], f32, tag="b1")
        nc.sync.dma_start(out=b1_t[:], in_=b1d[:])
        b2_t = wp.tile([128, 2], f32, tag="b2")
        nc.sync.dma_start(out=b2_t[:], in_=b2d[:])
        b3_t = wp.tile([128, 2], f32, tag="b3")
        nc.sync.dma_start(out=b3_t[:], in_=b3d[:])

        # raw per-point scores: partition p = b*NT + t holds points
        # [t*TN, (t+1)*TN) of bbox b
        score_sb = sp.tile([128, TN], f32, tag="score")

        # 4-stage software pipeline over global tile index j = b*NT + t:
        # iteration i runs L1(i), L2(i-1), L3(i-2), cls(i-3), so every
        # matmul's rhs was produced a full iteration (~3us) earlier and the
        # PE never waits on a same-iteration ReLU.
        TOT = B * NT
        xts, h1s, h2s, h3s = {}, {}, {}, {}
        psc_by_bbox = {}

        for i in range(TOT + 3):
            # stage 0: input DMA + layer 1 [6 -> 256] for tile i
            if i < TOT:
                b, t = divmod(i, NT)
                xt_t = xp.tile([IN_DIM, TN], f32r, tag="xt", bufs=6)
                nc.sync.dma_start(out=xt_t[:], in_=xt[b, :, t * TN:(t + 1) * TN])
                xts[i] = xt_t

                pa0 = pp.tile([128, TN], f32, tag="pa0")
                pa1 = pp.tile([128, TN], f32, tag="pa1")
                nc.tensor.matmul(pa0[:], w1_t[:, 0:128], xt_t[:], start=True, stop=True)
                nc.tensor.matmul(pa1[:], w1_t[:, 128:256], xt_t[:], start=True, stop=True)
                h10 = hp.tile([128, TN], f32r, tag="h10")
                h11 = hp.tile([128, TN], f32r, tag="h11")
                nc.scalar.activation(out=h10[:], in_=pa0[:], func=RELU, bias=b1_t[:, 0:1], scale=1.0)
                nc.vector.tensor_scalar(
                    out=h11[:], in0=pa1[:], scalar1=b1_t[:, 1:2], scalar2=0.0, op0=ADD, op1=MAX
                )
                h1s[i] = (h10, h11)

            # stage 1: layer 2 [256 -> 256] for tile i-1
            j = i - 1
            if 0 <= j < TOT:
                h10, h11 = h1s.pop(j)
                pb0 = pp.tile([128, TN], f32, tag="pb0")
                pb1 = pp.tile([128, TN], f32, tag="pb1")
                nc.tensor.matmul(pb0[:], w2_t[:, 0:128], h10[:], start=True, stop=False)
                nc.tensor.matmul(pb0[:], w2_t[:, 256:384], h11[:], start=False, stop=True)
                nc.tensor.matmul(pb1[:], w2_t[:, 128:256], h10[:], start=True, stop=False)
                nc.tensor.matmul(pb1[:], w2_t[:, 384:512], h11[:], start=False, stop=True)
                h20 = hp.tile([128, TN], f32r, tag="h20")
                h21 = hp.tile([128, TN], f32r, tag="h21")
                nc.scalar.activation(out=h20[:], in_=pb0[:], func=RELU, bias=b2_t[:, 0:1], scale=1.0)
                nc.vector.tensor_scalar(
                    out=h21[:], in0=pb1[:], scalar1=b2_t[:, 1:2], scalar2=0.0, op0=ADD, op1=MAX
                )
                h2s[j] = (h20, h21)

            # stage 2: layer 3 [256 -> 256] for tile i-2
            j = i - 2
            if 0 <= j < TOT:
                h20, h21 = h2s.pop(j)
                pc0 = pp.tile([128, TN], f32, tag="pc0")
                pc1 = pp.tile([128, TN], f32, tag="pc1")
                nc.tensor.matmul(pc0[:], w3_t[:, 0:128], h20[:], start=True, stop=False)
                nc.tensor.matmul(pc0[:], w3_t[:, 256:384], h21[:], start=False, stop=True)
                nc.tensor.matmul(pc1[:], w3_t[:, 128:256], h20[:], start=True, stop=False)
                nc.tensor.matmul(pc1[:], w3_t[:, 384:512], h21[:], start=False, stop=True)
                h30 = hp.tile([128, TN], f32r, tag="h30")
                h31 = hp.tile([128, TN], f32r, tag="h31")
                nc.scalar.activation(out=h30[:], in_=pc0[:], func=RELU, bias=b3_t[:, 0:1], scale=1.0)
                nc.vector.tensor_scalar(
                    out=h31[:], in0=pc1[:], scalar1=b3_t[:, 1:2], scalar2=0.0, op0=ADD, op1=MAX
                )
                h3s[j] = (h30, h31)

            # stage 3: classifier for tile i-3
            # masked-column weights put tile t's scores in psum row t of the
            # bbox's [16, 512] accumulator bank
            j = i - 3
            if 0 <= j < TOT:
                b, t = divmod(j, NT)
                if t == 0:
                    psc_by_bbox[b] = pp.tile([NT, TN], f32, tag="psc", bufs=2, name="psc")
                psc = psc_by_bbox[b]
                h30, h31 = h3s.pop(j)
                xt_t = xts.pop(j)
                nc.tensor.matmul(
                    psc[:], cw6_t[:, t * NT:(t + 1) * NT], xt_t[:],
                    start=(t == 0), stop=False)
                nc.tensor.matmul(
                    psc[:], cwh_t[:, t * 32:t * 32 + 16], h30[:],
                    start=False, stop=False)
                nc.tensor.matmul(
                    psc[:], cwh_t[:, t * 32 + 16:t * 32 + 32], h31[:],
                    start=False, stop=(t == NT - 1))

                if t == NT - 1:
                    # drain the bbox's [16, 512] scores: PSUM -> SBUF -> DMA
                    psc = psc_by_bbox.pop(b)
                    stg = hp.tile([NT, TN], f32, tag="stg", bufs=2)
                    if b % 2 == 0:
                        nc.scalar.copy(out=stg[:], in_=psc[:])
                    else:
                        nc.vector.tensor_copy(out=stg[:], in_=psc[:])
                    nc.sync.dma_start(out=score_sb[b * NT:(b + 1) * NT, :], in_=stg[:])

        # per-bbox min/max rescale tail
        mn = st.tile([128, 1], f32, tag="mn")
        mx = st.tile([128, 1], f32, tag="mx")
        nc.vector.tensor_reduce(out=mn[:], in_=score_sb[:], axis=AX, op=MIN)
        nc.vector.tensor_reduce(out=mx[:], in_=score_sb[:], axis=AX, op=MAX)

        # regroup [128,1] -> [8,16] via DRAM so each bbox's partials share a row
        mnd = dsc.tile([128], f32, tag="mnd")
        mxd = dsc.tile([128], f32, tag="mxd")
        nc.sync.dma_start(out=mnd[:], in_=mn[:])
        nc.sync.dma_start(out=mxd[:], in_=mx[:])
        mn8 = st.tile([B, NT], f32, tag="mn8")
        mx8 = st.tile([B, NT], f32, tag="mx8")
        nc.sync.dma_start(out=mn8[:], in_=mnd[:].rearrange("(a c) -> a c", a=B))
        nc.sync.dma_start(out=mx8[:], in_=mxd[:].rearrange("(a c) -> a c", a=B))
        mnb = st.tile([B, 1], f32, tag="mnb")
        mxb = st.tile([B, 1], f32, tag="mxb")
        nc.vector.tensor_reduce(out=mnb[:], in_=mn8[:], axis=AX, op=MIN)
        nc.vector.tensor_reduce(out=mxb[:], in_=mx8[:], axis=AX, op=MAX)

        # k2 = 2/(max-min); off = -min*k2 - 1
        rngb = st.tile([B, 1], f32, tag="rngb")
        nc.vector.tensor_tensor(out=rngb[:], in0=mxb[:], in1=mnb[:], op=SUB)
        invb = st.tile([B, 1], f32, tag="invb")
        nc.vector.reciprocal(out=invb[:], in_=rngb[:])
        k2b = st.tile([B, 1], f32, tag="k2b")
        nc.vector.tensor_scalar(
            out=k2b[:], in0=invb[:], scalar1=2.0, scalar2=None, op0=MULT
        )
        tmpb = st.tile([B, 1], f32, tag="tmpb")
        nc.vector.tensor_tensor(out=tmpb[:], in0=mnb[:], in1=k2b[:], op=MULT)
        offb = st.tile([B, 1], f32, tag="offb")
        nc.vector.tensor_scalar(
            out=offb[:], in0=tmpb[:], scalar1=-1.0, scalar2=-1.0, op0=MULT, op1=ADD
        )

        # broadcast [8,1] -> [128,1] (partition p gets value for bbox p//NT)
        k2d = dsc.tile([B], f32, tag="k2d")
        offd = dsc.tile([B], f32, tag="offd")
        nc.sync.dma_start(out=k2d[:], in_=k2b[:])
        nc.sync.dma_start(out=offd[:], in_=offb[:])
        k2f = st.tile([128, 1], f32, tag="k2f")
        offf = st.tile([128, 1], f32, tag="offf")
        k2d_ap = k2d[:]
        offd_ap = offd[:]
        nc.sync.dma_start(
            out=k2f[:],
            in_=bass.AP(tensor=k2d_ap.tensor, offset=k2d_ap.offset, ap=[[1, B], [0, NT]]),
        )
        nc.sync.dma_start(
            out=offf[:],
            in_=bass.AP(tensor=offd_ap.tensor, offset=offd_ap.offset, ap=[[1, B], [0, NT]]),
        )

        final = sp.tile([128, TN], f32, tag="final")
        nc.vector.tensor_scalar(
            out=final[:], in0=score_sb[:], scalar1=k2f[:], scalar2=offf[:], op0=MULT, op1=ADD
        )
        nc.sync.dma_start(out=score_d[:], in_=final[:])

    nc.finalize()
    return nc


def _get_program():
    if "nc" not in _CACHE:
        _CACHE["nc"] = _build_program()
    return _CACHE["nc"]


def _make_in_maps(np_inputs):
    feat = np.asarray(np_inputs["feat"], dtype=np.float32)
    eW1 = np.asarray(np_inputs["eW1"], dtype=np.float32)
    eb1 = np.asarray(np_inputs["eb1"], dtype=np.float32)
    eW2 = np.asarray(np_inputs["eW2"], dtype=np.float32)
    eb2 = np.asarray(np_inputs["eb2"], dtype=np.float32)
    eW3 = np.asarray(np_inputs["eW3"], dtype=np.float32)
    eb3 = np.asarray(np_inputs["eb3"], dtype=np.float32)
    cW = np.asarray(np_inputs["cW"], dtype=np.float32)

    # [64, 8192, 6] -> [64, 6, 8192], channel-major per bbox
    xt_all = np.ascontiguousarray(np.transpose(feat, (0, 2, 1)))

    def pack_w(w):  # [256,256] -> [128, 512] blocks [k0m0 | k0m1 | k1m0 | k1m1]
        return np.ascontiguousarray(np.concatenate(
            [w[0:128, 0:128], w[0:128, 128:256], w[128:256, 0:128], w[128:256, 128:256]],
            axis=1))

    # masked classifier weights: tile t's live column is t
    cw6m = np.zeros((IN_DIM, NT, NT), dtype=np.float32)
    cwhm = np.zeros((128, NT, 2 * NT), dtype=np.float32)
    for t in range(NT):
        cw6m[:, t, t] = cW[0:IN_DIM, 0]
        cwhm[:, t, t] = cW[6:134, 0]
        cwhm[:, t, NT + t] = cW[134:262, 0]

    common = {
        "w1": np.ascontiguousarray(eW1),
        "w2p": pack_w(eW2),
        "w3p": pack_w(eW3),
        "cw6m": np.ascontiguousarray(cw6m.reshape(IN_DIM, NT * NT)),
        "cwhm": np.ascontiguousarray(cwhm.reshape(128, NT * 2 * NT)),
        "b1": np.ascontiguousarray(np.stack([eb1[0:128], eb1[128:256]], axis=1)),
        "b2": np.ascontiguousarray(np.stack([eb2[0:128], eb2[128:256]], axis=1)),
        "b3": np.ascontiguousarray(np.stack([eb3[0:128], eb3[128:256]], axis=1)),
    }
    return [
        {"xt": np.ascontiguousarray(xt_all[c * B:(c + 1) * B]), **common}
        for c in range(N_CORES)
    ]


def kernel(feat, eW1, eb1, eW2, eb2, eW3, eb3,
           sW1, sb1, sW2, sb2, sW3, sb3, cW, cb):
    nc = _get_program()
    in_maps = _make_in_maps({
        "feat": feat, "eW1": eW1, "eb1": eb1, "eW2": eW2, "eb2": eb2,
        "eW3": eW3, "eb3": eb3, "cW": cW,
    })
    res = run_bass_kernel_spmd(nc, in_maps, list(range(N_CORES))).results
    out = np.concatenate(
        [np.asarray(res[c]["score"]).reshape(B, NPTS) for c in range(N_CORES)], axis=0
    )
    return np.ascontiguousarray(out.astype(np.float32))


# revision 12
# speedup vs baseline: 1.3697x; 1.1186x over previous
"""PointNet-style kernel for Trainium2, sharded across 8 NeuronCores.

Math note: reference computes score = [feat | hidden | seg_tiled] @ cW + cb,
then per-bbox rescales (s - min) / (max - min) * 2 - 1.  The seg-MLP
contribution and cb are constant within a bbox, and the rescale is invariant
to per-bbox additive constants, so they cancel exactly.  Only the embed MLP
and the per-point part of the classifier affect the output:
    s_pt[m, n] = feat[m,n,:] @ cW[0:6] + h3[m,n,:] @ cW[6:262]

Sharding: pure data-parallel over the M (bbox) axis - 8 bboxes per core.

Device layout per core: activations are kept channels-on-partitions,
points-streaming ([C, Npts] tiles of 512 points).  Matmuls run in float32r
(fp32 storage, reduced-precision multiply, 4x faster than fp32 on the PE,
~1.5e-4 rel err per matmul as measured on hardware).
"""

import numpy as np

import concourse.bass as bass
import concourse.mybir as mybir
import concourse.tile as tile
from concourse import bacc
from concourse.bass_utils import run_bass_kernel_spmd

N_CORES = 8
B = 8            # bboxes per core
NPTS = 8192      # points per bbox
IN_DIM = 6
HID = 256
TN = 512         # points per tile
NT = NPTS // TN  # 16 tiles per bbox

f32 = mybir.dt.float32
USE_BF16 = True
f32r = mybir.dt.bfloat16 if USE_BF16 else mybir.dt.float32r
RELU = mybir.ActivationFunctionType.Relu
ADD = mybir.AluOpType.add
MAX = mybir.AluOpType.max
MIN = mybir.AluOpType.min
MULT = mybir.AluOpType.mult
SUB = mybir.AluOpType.subtract
AX = mybir.AxisListType.X

_CACHE = {}


def _build_program():
    nc = bacc.Bacc("TRN2", target_bir_lowering=False, debug=False)

    xt = nc.dram_tensor("xt", [B, IN_DIM, NPTS], f32r, kind="ExternalInput")
    w1 = nc.dram_tensor("w1", [IN_DIM, HID], f32r, kind="ExternalInput")
    w2p = nc.dram_tensor("w2p", [128, 512], f32r, kind="ExternalInput")
    w3p = nc.dram_tensor("w3p", [128, 512], f32r, kind="ExternalInput")
    # classifier weights with masked columns: for tile t, column t (of 16) is
    # live and the rest are zero, so all 16 tiles of a bbox accumulate into
    # one [16, 512] PSUM bank (row t = tile t's scores)
    cw6m = nc.dram_tensor("cw6m", [IN_DIM, NT * NT], f32r, kind="ExternalInput")
    cwhm = nc.dram_tensor("cwhm", [128, NT * 2 * NT], f32r, kind="ExternalInput")
    b1d = nc.dram_tensor("b1", [128, 2], f32, kind="ExternalInput")
    b2d = nc.dram_tensor("b2", [128, 2], f32, kind="ExternalInput")
    b3d = nc.dram_tensor("b3", [128, 2], f32, kind="ExternalInput")
    score_d = nc.dram_tensor("score", [B, NT, TN], f32, kind="ExternalOutput")

    with (
        tile.TileContext(nc) as tc,
        tc.tile_pool(name="wp", bufs=1) as wp,
        tc.tile_pool(name="xp", bufs=4) as xp,
        tc.tile_pool(name="hp", bufs=3) as hp,
        tc.tile_pool(name="sp", bufs=1) as sp,
        tc.tile_pool(name="st", bufs=1) as st,
        tc.tile_pool(name="dsc", bufs=1, space="DRAM") as dsc,
        tc.tile_pool(name="pp", bufs=1, space="PSUM") as pp,
    ):
        w1_t = wp.tile([IN_DIM, HID], f32r, tag="w1")
        nc.sync.dma_start(out=w1_t[:], in_=w1[:])
        w2_t = wp.tile([128, 512], f32r, tag="w2")
        nc.sync.dma_start(out=w2_t[:], in_=w2p[:])
        w3_t = wp.tile([128, 512], f32r, tag="w3")
        nc.sync.dma_start(out=w3_t[:], in_=w3p[:])
        cw6_t = wp.tile([IN_DIM, NT * NT], f32r, tag="cw6")
        nc.sync.dma_start(out=cw6_t[:], in_=cw6m[:])
        cwh_t = wp.tile([128, NT * 2 * NT], f32r, tag="cwh")
        nc.sync.dma_start(out=cwh_t[:], in_=cwhm[:])
        b1_t = wp.tile([128, 2], f32, tag="b1")
        nc.sync.dma_start(out=b1_t[:], in_=b1d[:])
        b2_t = wp.tile([128, 2], f32, tag="b2")
        nc.sync.dma_start(out=b2_t[:], in_=b2d[:])
        b3_t = wp.tile([128, 2], f32, tag="b3")
        nc.sync.dma_start(out=b3_t[:], in_=b3d[:])

        # raw per-point scores: partition p = b*NT + t holds points
        # [t*TN, (t+1)*TN) of bbox b
        score_sb = sp.tile([128, TN], f32, tag="score")

        # 4-stage software pipeline over global tile index j = b*NT + t:
        # iteration i runs L1(i), L2(i-1), L3(i-2), cls(i-3), so every
        # matmul's rhs was produced a full iteration (~3us) earlier and the
        # PE never waits on a same-iteration ReLU.
        TOT = B * NT
        xts, h1s, h2s, h3s = {}, {}, {}, {}
        psc_by_bbox = {}

        for i in range(TOT + 3):
            # stage 0: input DMA + layer 1 [6 -> 256] for tile i
            if i < TOT:
                b, t = divmod(i, NT)
                xt_t = xp.tile([IN_DIM, TN], f32r, tag="xt", bufs=6)
                nc.sync.dma_start(out=xt_t[:], in_=xt[b, :, t * TN:(t + 1) * TN])
                xts[i] = xt_t

                pa0 = pp.tile([128, TN], f32, tag="pa0")
                pa1 = pp.tile([128, TN], f32, tag="pa1")
                nc.tensor.matmul(pa0[:], w1_t[:, 0:128], xt_t[:], start=True, stop=True)
                nc.tensor.matmul(pa1[:], w1_t[:, 128:256], xt_t[:], start=True, stop=True)
                h10 = hp.tile([128, TN], f32r, tag="h10")
                h11 = hp.tile([128, TN], f32r, tag="h11")
                nc.scalar.activation(out=h10[:], in_=pa0[:], func=RELU, bias=b1_t[:, 0:1], scale=1.0)
                nc.vector.tensor_scalar(
                    out=h11[:], in0=pa1[:], scalar1=b1_t[:, 1:2], scalar2=0.0, op0=ADD, op1=MAX
                )
                h1s[i] = (h10, h11)

            # stage 1: layer 2 [256 -> 256] for tile i-1
            j = i - 1
            if 0 <= j < TOT:
                h10, h11 = h1s.pop(j)
                pb0 = pp.tile([128, TN], f32, tag="pb0")
                pb1 = pp.tile([128, TN], f32, tag="pb1")
                nc.tensor.matmul(pb0[:], w2_t[:, 0:128], h10[:], start=True, stop=False)
                nc.tensor.matmul(pb0[:], w2_t[:, 256:384], h11[:], start=False, stop=True)
                nc.tensor.matmul(pb1[:], w2_t[:, 128:256], h10[:], start=True, stop=False)
                nc.tensor.matmul(pb1[:], w2_t[:, 384:512], h11[:], start=False, stop=True)
                h20 = hp.tile([128, TN], f32r, tag="h20")
                h21 = hp.tile([128, TN], f32r, tag="h21")
                nc.scalar.activation(out=h20[:], in_=pb0[:], func=RELU, bias=b2_t[:, 0:1], scale=1.0)
                nc.vector.tensor_scalar(
                    out=h21[:], in0=pb1[:], scalar1=b2_t[:, 1:2], scalar2=0.0, op0=ADD, op1=MAX
                )
                h2s[j] = (h20, h21)

            # stage 2: layer 3 [256 -> 256] for tile i-2
            j = i - 2
            if 0 <= j < TOT:
                h20, h21 = h2s.pop(j)
                pc0 = pp.tile([128, TN], f32, tag="pc0")
                pc1 = pp.tile([128, TN], f32, tag="pc1")
                nc.tensor.matmul(pc0[:], w3_t[:, 0:128], h20[:], start=True, stop=False)
                nc.tensor.matmul(pc0[:], w3_t[:, 256:384], h21[:], start=False, stop=True)
                nc.tensor.matmul(pc1[:], w3_t[:, 128:256], h20[:], start=True, stop=False)
                nc.tensor.matmul(pc1[:], w3_t[:, 384:512], h21[:], start=False, stop=True)
                h30 = hp.tile([128, TN], f32r, tag="h30")
                h31 = hp.tile([128, TN], f32r, tag="h31")
                nc.scalar.activation(out=h30[:], in_=pc0[:], func=RELU, bias=b3_t[:, 0:1], scale=1.0)
                nc.vector.tensor_scalar(
                    out=h31[:], in0=pc1[:], scalar1=b3_t[:, 1:2], scalar2=0.0, op0=ADD, op1=MAX
                )
                h3s[j] = (h30, h31)

            # stage 3: classifier for tile i-3
            # masked-column weights put tile t's scores in psum row t of the
            # bbox's [16, 512] accumulator bank
            j = i - 3
            if 0 <= j < TOT:
                b, t = divmod(j, NT)
                if t == 0:
                    psc_by_bbox[b] = pp.tile([NT, TN], f32, tag="psc", bufs=2, name="psc")
                psc = psc_by_bbox[b]
                h30, h31 = h3s.pop(j)
                xt_t = xts.pop(j)
                nc.tensor.matmul(
                    psc[:], cw6_t[:, t * NT:(t + 1) * NT], xt_t[:],
                    start=(t == 0), stop=False)
                nc.tensor.matmul(
                    psc[:], cwh_t[:, t * 32:t * 32 + 16], h30[:],
                    start=False, stop=False)
                nc.tensor.matmul(
                    psc[:], cwh_t[:, t * 32 + 16:t * 32 + 32], h31[:],
                    start=False, stop=(t == NT - 1))

                if t == NT - 1:
                    # drain the bbox's [16, 512] scores: PSUM -> SBUF -> DMA
                    psc = psc_by_bbox.pop(b)
                    stg = hp.tile([NT, TN], f32, tag="stg", bufs=2)
                    if b % 2 == 0:
                        nc.scalar.copy(out=stg[:], in_=psc[:])
                    else:
                        nc.vector.tensor_copy(out=stg[:], in_=psc[:])
                    nc.sync.dma_start(out=score_sb[b * NT:(b + 1) * NT, :], in_=stg[:])

        # per-bbox min/max rescale tail
        mn = st.tile([128, 1], f32, tag="mn")
        mx = st.tile([128, 1], f32, tag="mx")
        nc.vector.tensor_reduce(out=mn[:], in_=score_sb[:], axis=AX, op=MIN)
        nc.vector.tensor_reduce(out=mx[:], in_=score_sb[:], axis=AX, op=MAX)

        # regroup [128,1] -> [8,16] via DRAM so each bbox's partials share a row
        mnd = dsc.tile([128], f32, tag="mnd")
        mxd = dsc.tile([128], f32, tag="mxd")
        nc.sync.dma_start(out=mnd[:], in_=mn[:])
        nc.sync.dma_start(out=mxd[:], in_=mx[:])
        mn8 = st.tile([B, NT], f32, tag="mn8")
        mx8 = st.tile([B, NT], f32, tag="mx8")
        nc.sync.dma_start(out=mn8[:], in_=mnd[:].rearrange("(a c) -> a c", a=B))
        nc.sync.dma_start(out=mx8[:], in_=mxd[:].rearrange("(a c) -> a c", a=B))
        mnb = st.tile([B, 1], f32, tag="mnb")
        mxb = st.tile([B, 1], f32, tag="mxb")
        nc.vector.tensor_reduce(out=mnb[:], in_=mn8[:], axis=AX, op=MIN)
        nc.vector.tensor_reduce(out=mxb[:], in_=mx8[:], axis=AX, op=MAX)

        # k2 = 2/(max-min); off = -min*k2 - 1
        rngb = st.tile([B, 1], f32, tag="rngb")
        nc.vector.tensor_tensor(out=rngb[:], in0=mxb[:], in1=mnb[:], op=SUB)
        invb = st.tile([B, 1], f32, tag="invb")
        nc.vector.reciprocal(out=invb[:], in_=rngb[:])
        k2b = st.tile([B, 1], f32, tag="k2b")
        nc.vector.tensor_scalar(
            out=k2b[:], in0=invb[:], scalar1=2.0, scalar2=None, op0=MULT
        )
        tmpb = st.tile([B, 1], f32, tag="tmpb")
        nc.vector.tensor_tensor(out=tmpb[:], in0=mnb[:], in1=k2b[:], op=MULT)
        offb = st.tile([B, 1], f32, tag="offb")
        nc.vector.tensor_scalar(
            out=offb[:], in0=tmpb[:], scalar1=-1.0, scalar2=-1.0, op0=MULT, op1=ADD
        )

        # broadcast [8,1] -> [128,1] (partition p gets value for bbox p//NT)
        k2d = dsc.tile([B], f32, tag="k2d")
        offd = dsc.tile([B], f32, tag="offd")
        nc.sync.dma_start(out=k2d[:], in_=k2b[:])
        nc.sync.dma_start(out=offd[:], in_=offb[:])
        k2f = st.tile([128, 1], f32, tag="k2f")
        offf = st.tile([128, 1], f32, tag="offf")
        k2d_ap = k2d[:]
        offd_ap = offd[:]
        nc.sync.dma_start(
            out=k2f[:],
            in_=bass.AP(tensor=k2d_ap.tensor, offset=k2d_ap.offset, ap=[[1, B], [0, NT]]),
        )
        nc.sync.dma_start(
            out=offf[:],
            in_=bass.AP(tensor=offd_ap.tensor, offset=offd_ap.offset, ap=[[1, B], [0, NT]]),
        )

        final = sp.tile([128, TN], f32, tag="final")
        nc.vector.tensor_scalar(
            out=final[:], in0=score_sb[:], scalar1=k2f[:], scalar2=offf[:], op0=MULT, op1=ADD
        )
        nc.sync.dma_start(out=score_d[:], in_=final[:])

    nc.finalize()
    return nc


def _get_program():
    if "nc" not in _CACHE:
        _CACHE["nc"] = _build_program()
    return _CACHE["nc"]


def _make_in_maps(np_inputs):
    feat = np.asarray(np_inputs["feat"], dtype=np.float32)
    eW1 = np.asarray(np_inputs["eW1"], dtype=np.float32)
    eb1 = np.asarray(np_inputs["eb1"], dtype=np.float32)
    eW2 = np.asarray(np_inputs["eW2"], dtype=np.float32)
    eb2 = np.asarray(np_inputs["eb2"], dtype=np.float32)
    eW3 = np.asarray(np_inputs["eW3"], dtype=np.float32)
    eb3 = np.asarray(np_inputs["eb3"], dtype=np.float32)
    cW = np.asarray(np_inputs["cW"], dtype=np.float32)

    # [64, 8192, 6] -> [64, 6, 8192], channel-major per bbox
    xt_all = np.ascontiguousarray(np.transpose(feat, (0, 2, 1)))

    def pack_w(w):  # [256,256] -> [128, 512] blocks [k0m0 | k0m1 | k1m0 | k1m1]
        return np.ascontiguousarray(np.concatenate(
            [w[0:128, 0:128], w[0:128, 128:256], w[128:256, 0:128], w[128:256, 128:256]],
            axis=1))

    # masked classifier weights: tile t's live column is t
    cw6m = np.zeros((IN_DIM, NT, NT), dtype=np.float32)
    cwhm = np.zeros((128, NT, 2 * NT), dtype=np.float32)
    for t in range(NT):
        cw6m[:, t, t] = cW[0:IN_DIM, 0]
        cwhm[:, t, t] = cW[6:134, 0]
        cwhm[:, t, NT + t] = cW[134:262, 0]

    if USE_BF16:
        import ml_dtypes
        mm_t = ml_dtypes.bfloat16
    else:
        mm_t = np.float32

    common = {
        "w1": np.ascontiguousarray(eW1).astype(mm_t),
        "w2p": pack_w(eW2).astype(mm_t),
        "w3p": pack_w(eW3).astype(mm_t),
        "cw6m": np.ascontiguousarray(cw6m.reshape(IN_DIM, NT * NT)).astype(mm_t),
        "cwhm": np.ascontiguousarray(cwhm.reshape(128, NT * 2 * NT)).astype(mm_t),
        "b1": np.ascontiguousarray(np.stack([eb1[0:128], eb1[128:256]], axis=1)),
        "b2": np.ascontiguousarray(np.stack([eb2[0:128], eb2[128:256]], axis=1)),
        "b3": np.ascontiguousarray(np.stack([eb3[0:128], eb3[128:256]], axis=1)),
    }
    return [
        {"xt": np.ascontiguousarray(xt_all[c * B:(c + 1) * B]).astype(mm_t), **common}
        for c in range(N_CORES)
    ]


def kernel(feat, eW1, eb1, eW2, eb2, eW3, eb3,
           sW1, sb1, sW2, sb2, sW3, sb3, cW, cb):
    nc = _get_program()
    in_maps = _make_in_maps({
        "feat": feat, "eW1": eW1, "eb1": eb1, "eW2": eW2, "eb2": eb2,
        "eW3": eW3, "eb3": eb3, "cW": cW,
    })
    res = run_bass_kernel_spmd(nc, in_maps, list(range(N_CORES))).results
    out = np.concatenate(
        [np.asarray(res[c]["score"]).reshape(B, NPTS) for c in range(N_CORES)], axis=0
    )
    return np.ascontiguousarray(out.astype(np.float32))


# revision 16
# speedup vs baseline: 1.4487x; 1.0576x over previous
"""PointNet-style kernel for Trainium2, sharded across 8 NeuronCores.

Math note: reference computes score = [feat | hidden | seg_tiled] @ cW + cb,
then per-bbox rescales (s - min) / (max - min) * 2 - 1.  The seg-MLP
contribution and cb are constant within a bbox, and the rescale is invariant
to per-bbox additive constants, so they cancel exactly.  Only the embed MLP
and the per-point part of the classifier affect the output:
    s_pt[m, n] = feat[m,n,:] @ cW[0:6] + h3[m,n,:] @ cW[6:262]

Sharding: pure data-parallel over the M (bbox) axis - 8 bboxes per core.

Device layout per core: activations are kept channels-on-partitions,
points-streaming ([C, Npts] tiles of 512 points).  Matmuls run in float32r
(fp32 storage, reduced-precision multiply, 4x faster than fp32 on the PE,
~1.5e-4 rel err per matmul as measured on hardware).
"""

import numpy as np

import concourse.bass as bass
import concourse.mybir as mybir
import concourse.tile as tile
from concourse import bacc
from concourse.bass_utils import run_bass_kernel_spmd

N_CORES = 8
B = 8            # bboxes per core
NPTS = 8192      # points per bbox
IN_DIM = 6
HID = 256
TN = 512         # points per tile
NT = NPTS // TN  # 16 tiles per bbox

f32 = mybir.dt.float32
USE_BF16 = True
f32r = mybir.dt.bfloat16 if USE_BF16 else mybir.dt.float32r
RELU = mybir.ActivationFunctionType.Relu
ADD = mybir.AluOpType.add
MAX = mybir.AluOpType.max
MIN = mybir.AluOpType.min
MULT = mybir.AluOpType.mult
SUB = mybir.AluOpType.subtract
AX = mybir.AxisListType.X

_CACHE = {}


def _build_program():
    nc = bacc.Bacc("TRN2", target_bir_lowering=False, debug=False)

    xt = nc.dram_tensor("xt", [B, IN_DIM, NPTS], f32r, kind="ExternalInput")
    w1 = nc.dram_tensor("w1", [IN_DIM, HID], f32r, kind="ExternalInput")
    w2p = nc.dram_tensor("w2p", [128, 512], f32r, kind="ExternalInput")
    w3p = nc.dram_tensor("w3p", [128, 512], f32r, kind="ExternalInput")
    # classifier weights with masked columns: for tile t, column t (of 16) is
    # live and the rest are zero, so all 16 tiles of a bbox accumulate into
    # one [16, 512] PSUM bank (row t = tile t's scores)
    cw6m = nc.dram_tensor("cw6m", [IN_DIM, NT * NT], f32r, kind="ExternalInput")
    cwhm = nc.dram_tensor("cwhm", [128, NT * 2 * NT], f32r, kind="ExternalInput")
    b1d = nc.dram_tensor("b1", [128, 2], f32, kind="ExternalInput")
    b2d = nc.dram_tensor("b2", [128, 2], f32, kind="ExternalInput")
    b3d = nc.dram_tensor("b3", [128, 2], f32, kind="ExternalInput")
    score_d = nc.dram_tensor("score", [B, NT, TN], f32, kind="ExternalOutput")

    with (
        tile.TileContext(nc) as tc,
        tc.tile_pool(name="wp", bufs=1) as wp,
        tc.tile_pool(name="xp", bufs=4) as xp,
        tc.tile_pool(name="hp", bufs=3) as hp,
        tc.tile_pool(name="sp", bufs=1) as sp,
        tc.tile_pool(name="st", bufs=1) as st,
        tc.tile_pool(name="dsc", bufs=1, space="DRAM") as dsc,
        tc.tile_pool(name="pp", bufs=1, space="PSUM") as pp,
    ):
        # W1 halves packed into PE row-groups 0 and 1 (partitions 0-5, 32-37)
        # so both L1 M-halves run as concurrent row-tiled matmuls; the masked
        # classifier feat weights sit in row-group 2 (partitions 64-69) and
        # ride along concurrently, accumulating straight into the psc bank.
        w1s = wp.tile([38, 128], f32r, tag="w1s")
        nc.sync.dma_start(out=w1s[0:IN_DIM, :], in_=w1[:, 0:128])
        nc.sync.dma_start(out=w1s[32:32 + IN_DIM, :], in_=w1[:, 128:256])
        cw6s = wp.tile([70, NT * NT], f32r, tag="cw6s")
        nc.sync.dma_start(out=cw6s[64:64 + IN_DIM, :], in_=cw6m[:])
        w2_t = wp.tile([128, 512], f32r, tag="w2")
        nc.sync.dma_start(out=w2_t[:], in_=w2p[:])
        w3_t = wp.tile([128, 512], f32r, tag="w3")
        nc.sync.dma_start(out=w3_t[:], in_=w3p[:])
        cwh_t = wp.tile([128, NT * 2 * NT], f32r, tag="cwh")
        nc.sync.dma_start(out=cwh_t[:], in_=cwhm[:])
        b1_t = wp.tile([128, 2], f32, tag="b1")
        nc.sync.dma_start(out=b1_t[:], in_=b1d[:])
        b2_t = wp.tile([128, 2], f32, tag="b2")
        nc.sync.dma_start(out=b2_t[:], in_=b2d[:])
        b3_t = wp.tile([128, 2], f32, tag="b3")
        nc.sync.dma_start(out=b3_t[:], in_=b3d[:])

        # raw per-point scores: partition p = b*NT + t holds points
        # [t*TN, (t+1)*TN) of bbox b
        score_sb = sp.tile([128, TN], f32, tag="score")

        # 4-stage software pipeline over global tile index j = b*NT + t:
        # iteration i runs L1(i), L2(i-1), L3(i-2), cls(i-3), so every
        # matmul's rhs was produced a full iteration (~3us) earlier and the
        # PE never waits on a same-iteration ReLU.
        TOT = B * NT
        h1s, h2s, h3s = {}, {}, {}
        psc_by_bbox = {}

        for i in range(TOT + 3):
            # stage 0: input DMA + layer 1 [6 -> 256] for tile i, both
            # M-halves as concurrent row-tiled matmuls (row groups 0/1), plus
            # the classifier feat-part riding row group 2 into the psc bank
            if i < TOT:
                b, t = divmod(i, NT)
                xtb = xp.tile([70, TN], f32r, tag="xt", bufs=6)
                src = xt[b, :, t * TN:(t + 1) * TN]
                nc.sync.dma_start(out=xtb[0:IN_DIM, :], in_=src)
                nc.sync.dma_start(out=xtb[32:32 + IN_DIM, :], in_=src)
                nc.sync.dma_start(out=xtb[64:64 + IN_DIM, :], in_=src)

                if t == 0:
                    psc_by_bbox[b] = pp.tile([NT, TN], f32, tag="psc", bufs=2, name="psc")
                psc = psc_by_bbox[b]

                pa0 = pp.tile([128, TN], f32, tag="pa0")
                pa1 = pp.tile([128, TN], f32, tag="pa1")
                nc.tensor.matmul(pa0[:], w1s[0:IN_DIM, :], xtb[0:IN_DIM, :],
                                 start=True, stop=True)
                nc.tensor.matmul(pa1[:], w1s[32:32 + IN_DIM, :], xtb[32:32 + IN_DIM, :],
                                 start=True, stop=True)
                nc.tensor.matmul(psc[:], cw6s[64:64 + IN_DIM, t * NT:(t + 1) * NT],
                                 xtb[64:64 + IN_DIM, :], start=(t == 0), stop=False)
                h10 = hp.tile([128, TN], f32r, tag="h10")
                h11 = hp.tile([128, TN], f32r, tag="h11")
                nc.scalar.activation(out=h10[:], in_=pa0[:], func=RELU, bias=b1_t[:, 0:1], scale=1.0)
                nc.vector.tensor_scalar(
                    out=h11[:], in0=pa1[:], scalar1=b1_t[:, 1:2], scalar2=0.0, op0=ADD, op1=MAX
                )
                h1s[i] = (h10, h11)

            # stage 1: layer 2 [256 -> 256] for tile i-1
            j = i - 1
            if 0 <= j < TOT:
                h10, h11 = h1s.pop(j)
                pb0 = pp.tile([128, TN], f32, tag="pb0")
                pb1 = pp.tile([128, TN], f32, tag="pb1")
                nc.tensor.matmul(pb0[:], w2_t[:, 0:128], h10[:], start=True, stop=False)
                nc.tensor.matmul(pb0[:], w2_t[:, 256:384], h11[:], start=False, stop=True)
                nc.tensor.matmul(pb1[:], w2_t[:, 128:256], h10[:], start=True, stop=False)
                nc.tensor.matmul(pb1[:], w2_t[:, 384:512], h11[:], start=False, stop=True)
                h20 = hp.tile([128, TN], f32r, tag="h20")
                h21 = hp.tile([128, TN], f32r, tag="h21")
                nc.scalar.activation(out=h20[:], in_=pb0[:], func=RELU, bias=b2_t[:, 0:1], scale=1.0)
                nc.vector.tensor_scalar(
                    out=h21[:], in0=pb1[:], scalar1=b2_t[:, 1:2], scalar2=0.0, op0=ADD, op1=MAX
                )
                h2s[j] = (h20, h21)

            # stage 2: layer 3 [256 -> 256] for tile i-2
            j = i - 2
            if 0 <= j < TOT:
                h20, h21 = h2s.pop(j)
                pc0 = pp.tile([128, TN], f32, tag="pc0")
                pc1 = pp.tile([128, TN], f32, tag="pc1")
                nc.tensor.matmul(pc0[:], w3_t[:, 0:128], h20[:], start=True, stop=False)
                nc.tensor.matmul(pc0[:], w3_t[:, 256:384], h21[:], start=False, stop=True)
                nc.tensor.matmul(pc1[:], w3_t[:, 128:256], h20[:], start=True, stop=False)
                nc.tensor.matmul(pc1[:], w3_t[:, 384:512], h21[:], start=False, stop=True)
                h30 = hp.tile([128, TN], f32r, tag="h30")
                h31 = hp.tile([128, TN], f32r, tag="h31")
                nc.scalar.activation(out=h30[:], in_=pc0[:], func=RELU, bias=b3_t[:, 0:1], scale=1.0)
                nc.vector.tensor_scalar(
                    out=h31[:], in0=pc1[:], scalar1=b3_t[:, 1:2], scalar2=0.0, op0=ADD, op1=MAX
                )
                h3s[j] = (h30, h31)

            # stage 3: classifier hidden-part for tile i-3
            # masked-column weights put tile t's scores in psum row t of the
            # bbox's [16, 512] accumulator bank (feat part already added by
            # the stage-0 free-rider)
            j = i - 3
            if 0 <= j < TOT:
                b, t = divmod(j, NT)
                psc = psc_by_bbox[b]
                h30, h31 = h3s.pop(j)
                nc.tensor.matmul(
                    psc[:], cwh_t[:, t * 32:t * 32 + 16], h30[:],
                    start=False, stop=False)
                nc.tensor.matmul(
                    psc[:], cwh_t[:, t * 32 + 16:t * 32 + 32], h31[:],
                    start=False, stop=(t == NT - 1))

                if t == NT - 1:
                    # drain the bbox's [16, 512] scores: PSUM -> SBUF -> DMA
                    psc = psc_by_bbox.pop(b)
                    stg = hp.tile([NT, TN], f32, tag="stg", bufs=2)
                    if b % 2 == 0:
                        nc.scalar.copy(out=stg[:], in_=psc[:])
                    else:
                        nc.vector.tensor_copy(out=stg[:], in_=psc[:])
                    nc.sync.dma_start(out=score_sb[b * NT:(b + 1) * NT, :], in_=stg[:])

        # per-bbox min/max rescale tail
        mn = st.tile([128, 1], f32, tag="mn")
        mx = st.tile([128, 1], f32, tag="mx")
        nc.vector.tensor_reduce(out=mn[:], in_=score_sb[:], axis=AX, op=MIN)
        nc.vector.tensor_reduce(out=mx[:], in_=score_sb[:], axis=AX, op=MAX)

        # regroup [128,1] -> [8,16] via DRAM so each bbox's partials share a row
        mnd = dsc.tile([128], f32, tag="mnd")
        mxd = dsc.tile([128], f32, tag="mxd")
        nc.sync.dma_start(out=mnd[:], in_=mn[:])
        nc.sync.dma_start(out=mxd[:], in_=mx[:])
        mn8 = st.tile([B, NT], f32, tag="mn8")
        mx8 = st.tile([B, NT], f32, tag="mx8")
        nc.sync.dma_start(out=mn8[:], in_=mnd[:].rearrange("(a c) -> a c", a=B))
        nc.sync.dma_start(out=mx8[:], in_=mxd[:].rearrange("(a c) -> a c", a=B))
        mnb = st.tile([B, 1], f32, tag="mnb")
        mxb = st.tile([B, 1], f32, tag="mxb")
        nc.vector.tensor_reduce(out=mnb[:], in_=mn8[:], axis=AX, op=MIN)
        nc.vector.tensor_reduce(out=mxb[:], in_=mx8[:], axis=AX, op=MAX)

        # k2 = 2/(max-min); off = -min*k2 - 1
        rngb = st.tile([B, 1], f32, tag="rngb")
        nc.vector.tensor_tensor(out=rngb[:], in0=mxb[:], in1=mnb[:], op=SUB)
        invb = st.tile([B, 1], f32, tag="invb")
        nc.vector.reciprocal(out=invb[:], in_=rngb[:])
        k2b = st.tile([B, 1], f32, tag="k2b")
        nc.vector.tensor_scalar(
            out=k2b[:], in0=invb[:], scalar1=2.0, scalar2=None, op0=MULT
        )
        tmpb = st.tile([B, 1], f32, tag="tmpb")
        nc.vector.tensor_tensor(out=tmpb[:], in0=mnb[:], in1=k2b[:], op=MULT)
        offb = st.tile([B, 1], f32, tag="offb")
        nc.vector.tensor_scalar(
            out=offb[:], in0=tmpb[:], scalar1=-1.0, scalar2=-1.0, op0=MULT, op1=ADD
        )

        # broadcast [8,1] -> [128,1] (partition p gets value for bbox p//NT)
        k2d = dsc.tile([B], f32, tag="k2d")
        offd = dsc.tile([B], f32, tag="offd")
        nc.sync.dma_start(out=k2d[:], in_=k2b[:])
        nc.sync.dma_start(out=offd[:], in_=offb[:])
        k2f = st.tile([128, 1], f32, tag="k2f")
        offf = st.tile([128, 1], f32, tag="offf")
        k2d_ap = k2d[:]
        offd_ap = offd[:]
        nc.sync.dma_start(
            out=k2f[:],
            in_=bass.AP(tensor=k2d_ap.tensor, offset=k2d_ap.offset, ap=[[1, B], [0, NT]]),
        )
        nc.sync.dma_start(
            out=offf[:],
            in_=bass.AP(tensor=offd_ap.tensor, offset=offd_ap.offset, ap=[[1, B], [0, NT]]),
        )

        final = sp.tile([128, TN], f32, tag="final")
        nc.vector.tensor_scalar(
            out=final[:], in0=score_sb[:], scalar1=k2f[:], scalar2=offf[:], op0=MULT, op1=ADD
        )
        nc.sync.dma_start(out=score_d[:], in_=final[:])

    nc.finalize()
    return nc


def _get_program():
    if "nc" not in _CACHE:
        _CACHE["nc"] = _build_program()
    return _CACHE["nc"]


def _make_in_maps(np_inputs):
    feat = np.asarray(np_inputs["feat"], dtype=np.float32)
    eW1 = np.asarray(np_inputs["eW1"], dtype=np.float32)
    eb1 = np.asarray(np_inputs["eb1"], dtype=np.float32)
    eW2 = np.asarray(np_inputs["eW2"], dtype=np.float32)
    eb2 = np.asarray(np_inputs["eb2"], dtype=np.float32)
    eW3 = np.asarray(np_inputs["eW3"], dtype=np.float32)
    eb3 = np.asarray(np_inputs["eb3"], dtype=np.float32)
    cW = np.asarray(np_inputs["cW"], dtype=np.float32)

    # [64, 8192, 6] -> [64, 6, 8192], channel-major per bbox
    xt_all = np.ascontiguousarray(np.transpose(feat, (0, 2, 1)))

    def pack_w(w):  # [256,256] -> [128, 512] blocks [k0m0 | k0m1 | k1m0 | k1m1]
        return np.ascontiguousarray(np.concatenate(
            [w[0:128, 0:128], w[0:128, 128:256], w[128:256, 0:128], w[128:256, 128:256]],
            axis=1))

    # masked classifier weights: tile t's live column is t
    cw6m = np.zeros((IN_DIM, NT, NT), dtype=np.float32)
    cwhm = np.zeros((128, NT, 2 * NT), dtype=np.float32)
    for t in range(NT):
        cw6m[:, t, t] = cW[0:IN_DIM, 0]
        cwhm[:, t, t] = cW[6:134, 0]
        cwhm[:, t, NT + t] = cW[134:262, 0]

    if USE_BF16:
        import ml_dtypes
        mm_t = ml_dtypes.bfloat16
    else:
        mm_t = np.float32

    common = {
        "w1": np.ascontiguousarray(eW1).astype(mm_t),
        "w2p": pack_w(eW2).astype(mm_t),
        "w3p": pack_w(eW3).astype(mm_t),
        "cw6m": np.ascontiguousarray(cw6m.reshape(IN_DIM, NT * NT)).astype(mm_t),
        "cwhm": np.ascontiguousarray(cwhm.reshape(128, NT * 2 * NT)).astype(mm_t),
        "b1": np.ascontiguousarray(np.stack([eb1[0:128], eb1[128:256]], axis=1)),
        "b2": np.ascontiguousarray(np.stack([eb2[0:128], eb2[128:256]], axis=1)),
        "b3": np.ascontiguousarray(np.stack([eb3[0:128], eb3[128:256]], axis=1)),
    }
    return [
        {"xt": np.ascontiguousarray(xt_all[c * B:(c + 1) * B]).astype(mm_t), **common}
        for c in range(N_CORES)
    ]


def kernel(feat, eW1, eb1, eW2, eb2, eW3, eb3,
           sW1, sb1, sW2, sb2, sW3, sb3, cW, cb):
    nc = _get_program()
    in_maps = _make_in_maps({
        "feat": feat, "eW1": eW1, "eb1": eb1, "eW2": eW2, "eb2": eb2,
        "eW3": eW3, "eb3": eb3, "cW": cW,
    })
    res = run_bass_kernel_spmd(nc, in_maps, list(range(N_CORES))).results
    out = np.concatenate(
        [np.asarray(res[c]["score"]).reshape(B, NPTS) for c in range(N_CORES)], axis=0
    )
    return np.ascontiguousarray(out.astype(np.float32))
